# revision 11
# baseline (speedup 1.0000x reference)
"""MinibatchDiscrimination Bass kernel for 8 TRN2 NeuronCores.

out[i,o] = sum_{j!=i} exp(-sum_k |M[i,k,o]-M[j,k,o]|),  M = x @ T.

Cyclic-offset pairing: shift t pairs row i with row (i+t) mod B; core c
computes t in [16c+1, 16c+16] (t=128 halved via an exp bias of -ln2),
covering every unordered pair exactly once.

Engine split per slot (t = t0 + s):
  - DVE (fp16 2x) computes max(M[i], M[i+t]) for 32 of the 38 ko-chunks
    in one merged instruction; the L1 identity
        sum|a-b| = 2*sum max(a,b) - sum a - sum b
    turns the PE reduction over those chunks + two "combo" chunks
    (partial row-sum corrections, stream-B total) into stream A
    (selector weights, col groups 0-1).
  - GPS computes a-b (TT subtract; this toolchain has no Pool max) and
    ACT abs() for the remaining 6 chunks; they are reduced directly by
    stream B (0.5-weighted selector, col groups 2-3) with no correction.
  - exp on ACT; e rows 64-127 stay zero so the e-accumulation matmuls
    reuse the selector weights (no PE weight swaps in phase 2 at all).
Shifted-operand alignment: mtb2 = M^T shifted by t0 (272 wide) makes
odd-t DVE reads 4B-aligned static and all GPS reads static; even-t DVE
reads use a dynamic aligned offset of mta. A PE warm-up burst during
the DMA lead-in keeps HAM at 8/8 through phase 1.
"""

import numpy as np

B = 256
F = 512
K = 75
O = 64
KO = K * O          # 4800
KOP = 4864          # padded to 38*128
NCH = KOP // 128    # 38 ko-chunks
NH = NCH // 2       # 19 chunks per half
CWA = 388           # mta chunk width (256 + 129 wrap + pad)
CWB = 272           # mtb2 chunk width: M^T[.., t0 + j], j in [0,272)
NSLOT = 16
NGPS = 3            # chunks per half on GPS+ACT abs path (when active)
GPS_START = 3       # first processed-slot index using the abs path
NDVE = NH - NGPS    # 16 chunks per half on DVE
LN2 = float(np.log(2.0))

_NC_CACHE = {}


def _build_nc():
    import concourse.bacc as bacc
    import concourse.bass as bass
    import concourse.mybir as mybir
    from concourse import tile

    fp16 = mybir.dt.float16
    fp32 = mybir.dt.float32
    bf16 = mybir.dt.bfloat16
    i32 = mybir.dt.int32
    Alu = mybir.AluOpType
    Act = mybir.ActivationFunctionType

    nc = bacc.Bacc(
        "TRN2", target_bir_lowering=False, debug=False, num_devices=8
    )

    with tile.TileContext(nc) as tc:
        xt_d = nc.dram_tensor("xt", [128, 1024], fp16, kind="ExternalInput")
        tt_d = nc.dram_tensor("tt", [128, NCH * 512], fp16, kind="ExternalInput")
        ts_d = nc.dram_tensor("tts2", [128, 512], fp16, kind="ExternalInput")
        ss_d = nc.dram_tensor("ssel", [128, 128], fp16, kind="ExternalInput")
        of_d = nc.dram_tensor("offs", [1, 2], i32, kind="ExternalInput")
        bi_d = nc.dram_tensor("bias2", [64, NSLOT], fp32, kind="ExternalInput")
        out_d = nc.dram_tensor("out", [64, 256], fp32, kind="ExternalOutput")

        with (
            tc.tile_pool(name="const", bufs=1) as cpool,
            tc.tile_pool(name="tload", bufs=3) as tpool,
            tc.tile_pool(name="mx", bufs=3) as mxpool,
            tc.tile_pool(name="mg", bufs=2) as mgpool,
            tc.tile_pool(name="cmb", bufs=2) as cpool2,
            tc.tile_pool(name="mpsum", bufs=2, space="PSUM") as mpsum,
            tc.tile_pool(name="dpa", bufs=2, space="PSUM") as dpapool,
            tc.tile_pool(name="dpb", bufs=2, space="PSUM") as dpbpool,
            tc.tile_pool(name="apsum", bufs=1, space="PSUM") as apsum,
        ):
            # small inputs first so the PE warm-up can start early
            ss2 = cpool.tile([128, 128], fp16)
            nc.sync.dma_start(ss2[:, :], ss_d[:, :])
            offs = cpool.tile([1, 2], i32)
            nc.sync.dma_start(offs[:, :], of_d[:, :])
            bias2 = cpool.tile([64, NSLOT], fp32)
            nc.sync.dma_start(bias2[:, :], bi_d[:, :])
            tts2 = cpool.tile([128, 512], fp16)
            nc.sync.dma_start(tts2[:, :], ts_d[:, :])
            xt = cpool.tile([128, 1024], fp16)
            for cc in range(4):
                nc.sync.dma_start(
                    xt[:, cc * 256 : (cc + 1) * 256],
                    xt_d[:, cc * 256 : (cc + 1) * 256],
                )
            tsb0 = tpool.tile([128, 1024], fp16, tag="tsb")
            nc.sync.dma_start(tsb0[:, :], tt_d[:, 0:1024])

            ssA = ss2[:, 0:64]     # 0/1 selector
            ssB = ss2[:, 64:128]   # 0.5-weighted selector

            # single M^T tile (subtile deps track chunk completion)
            mta = cpool.tile([128, NCH * CWA], fp16, name="mta", tag="mta")
            mtb = cpool.tile([128, NCH * CWB], fp16, name="mtb", tag="mtb")
            mta3 = mta[:, :].rearrange("p (c w) -> p c w", w=CWA)
            mtb3 = mtb[:, :].rearrange("p (c w) -> p c w", w=CWB)

            # acc bank: sa (early) + ps_self (late) share one psum bank
            acc = apsum.tile([64, 512], fp32, tag="acc")
            sa_ps = acc[:, 0:256]
            ps_self = acc[:, 256:512]
            ps_pair = apsum.tile([64, 512], fp32, tag="ppair")
            nc.vector.memset(ps_pair[:, :], 0.0)

            # ---- PE warm-up during the DMA lead-in (HAM -> 8/8) ----
            warm = mpsum.tile([128, 512], fp32, tag="mp")
            for w in range(64):
                nc.tensor.matmul(
                    warm[0:64, 0:64],
                    ssA,
                    ssA,
                    start=(w == 0),
                    stop=(w == 63),
                )

            # row-sums from host-presummed T: partial (DVE chunk k-set)
            # then full; each group: sa[o,i] = sum_f TS[f,o] * x[i,f]
            tts3 = tts2[:, :].rearrange("p (g c w) -> p g c w", g=2, w=64)
            csa2 = []
            for g in range(2):  # 0 = partial, 1 = full
                for cc in range(4):
                    nc.tensor.matmul(
                        sa_ps,
                        tts3[:, g, cc, :],
                        xt[:, cc * 256 : (cc + 1) * 256],
                        start=(cc == 0),
                        stop=(cc == 3),
                    )
                cs = cpool.tile([64, 512], fp16, name=f"csa2_{g}")
                nc.scalar.activation(cs[:, 0:256], sa_ps, Act.Copy, scale=-0.5)
                nc.scalar.activation(cs[:, 256:512], sa_ps, Act.Copy, scale=-0.5)
                csa2.append(cs)

            # static combo chunks (rows 0-63 = csa_i, rows 64-127 = 0)
            comboS = []
            for g in range(2):
                cb = cpool.tile([128, 256], fp16, name=f"comboS_{g}")
                nc.vector.memset(cb[64:128, :], 0.0)
                nc.scalar.copy(cb[0:64, :], csa2[g][:, 0:256])
                comboS.append(cb)
            # comboZ: per-slot dynamic corr_j for slots with no B stream
            comboZ = []
            for z in range(2):
                cb = cpool.tile([128, 256], fp16, name=f"comboZ_{z}")
                nc.vector.memset(cb[64:128, :], 0.0)
                comboZ.append(cb)

            # e tiles: rows 64-127 stay zero forever (selector e-acc)
            e_bufs = [cpool.tile([128, 256], bf16, name=f"e{i}") for i in range(3)]
            for eb in e_bufs:
                nc.vector.memset(eb[64:128, :], 0.0)

            # t0 registers per engine
            rtv = nc.vector.alloc_register("t0v")
            nc.vector.reg_load(rtv, offs[0:1, 0:1])
            vt0 = nc.vector.snap(rtv, donate=True, min_val=1, max_val=113)
            rts = nc.scalar.alloc_register("t0s")
            nc.scalar.reg_load(rts, offs[0:1, 0:1])
            st0 = nc.scalar.snap(rts, donate=True, min_val=1, max_val=113)
            rtg = nc.gpsimd.alloc_register("t0g")
            nc.gpsimd.reg_load(rtg, offs[0:1, 0:1])
            gt0 = nc.gpsimd.snap(rtg, donate=True, min_val=1, max_val=113)
            rtp = nc.tensor.alloc_register("t0p")
            nc.tensor.reg_load(rtp, offs[0:1, 0:1])
            vp0 = nc.tensor.snap(rtp, donate=True, min_val=1, max_val=113)

            # chunk index helpers
            # DVE chunks: [0,16) and [19,35); abs chunks: [16,19), [35,38)
            ND2 = NDVE  # 16

            order = [s for s in range(NSLOT) if s % 2 == 1] + [
                s for s in range(NSLOT) if s % 2 == 0
            ]
            recs = []

            def make_rec(si, s):
                use_abs = si >= GPS_START
                mx = mxpool.tile([128, NCH * 256], fp16, name="mx", tag="mx")
                m3 = mx[:, :].rearrange("p (c w) -> p c w", w=256)
                mg = None
                if use_abs:
                    mg = mgpool.tile(
                        [128, 2 * NGPS * 256], fp16, name="mg", tag="mg"
                    )
                return (si, s, use_abs, m3, mg)

            def dve_aps(h_groups, s, width, src_mta):
                """4D views over chunk groups for DVE in/out."""
                pass

            def emit_dve(rec, h):
                """Emit DVE max for half h (0/1). For full-width slots
                (no abs path) covers 19 chunks, else 16."""
                si, s, use_abs, m3, mg = rec
                nck = ND2 if use_abs else NH
                c0 = h * NH
                if s % 2 == 0:  # odd t: static aligned read of mtb2
                    src = mtb3[:, c0 : c0 + nck, s : s + 256]
                else:  # even t: dynamic aligned read of mta
                    src = mta3[:, c0 : c0 + nck, bass.ds(vt0 + s, 256)]
                nc.vector.tensor_tensor(
                    m3[:, c0 : c0 + nck, :],
                    mta3[:, c0 : c0 + nck, 0:256],
                    src,
                    Alu.max,
                )

            def emit_gps(rec, h):
                si, s, use_abs, m3, mg = rec
                if not use_abs:
                    return
                c0 = h * NH + ND2
                mg3 = mg[:, :].rearrange("p (g c w) -> p g c w", g=2, w=256)
                nc.gpsimd.tensor_tensor(
                    mg3[:, h, :, :],
                    mta3[:, c0 : c0 + NGPS, 0:256],
                    mtb3[:, c0 : c0 + NGPS, s : s + 256],
                    Alu.subtract,
                )

            def emit_abs(rec, h):
                si, s, use_abs, m3, mg = rec
                if not use_abs:
                    return
                c0 = h * NH + ND2
                mg3 = mg[:, :].rearrange("p (g c w) -> p g c w", g=2, w=256)
                nc.scalar.activation(
                    m3[:, c0 : c0 + NGPS, :], mg3[:, h, :, :], Act.Abs
                )

            def emit_pe(rec):
                si, s, use_abs, m3, mg = rec
                gsel = 0 if use_abs else 1  # which csa/comboS set
                dpa_t = dpapool.tile([128, 512], fp32, name="dpa_t", tag="dpa")
                dpa = dpa_t[:, 0:256]
                # stream A: DVE max chunks + combos (col groups 0-1)
                # stream B: abs chunks with 0.5 selector (col groups 2-3)
                a_chunks = (
                    list(range(0, ND2)) + list(range(NH, NH + ND2))
                    if use_abs
                    else list(range(0, NCH))
                )
                b_chunks = (
                    [ND2 + i for i in range(NGPS)]
                    + [NH + ND2 + i for i in range(NGPS)]
                    if use_abs
                    else []
                )
                if use_abs:
                    dpb_t = dpbpool.tile(
                        [128, 512], fp32, name="dpb_t", tag="dpb"
                    )
                    dpb = dpb_t[:, 0:256]
                nb = len(b_chunks)
                boff = 16  # abs results land ~1.7us after the DVE max
                for ci, c in enumerate(a_chunks):
                    nc.tensor.matmul(
                        dpa[0:64, :],
                        ssA,
                        m3[:, c, :],
                        start=(ci == 0),
                        stop=False,
                        tile_position=(0, 0),
                    )
                    bi = ci - boff
                    if 0 <= bi < nb:
                        nc.tensor.matmul(
                            dpb[64:128, :],
                            ssB,
                            m3[:, b_chunks[bi], :],
                            start=(bi == 0),
                            stop=(bi == nb - 1),
                            tile_position=(0, 64),
                        )
                nc.tensor.matmul(
                    dpa[0:64, :],
                    ssA,
                    comboS[gsel][:, :],
                    start=False,
                    stop=False,
                    tile_position=(0, 0),
                )
                # comboF/comboZ: rows 0-63 = csa_j, rows 64-127 = B total
                if use_abs:
                    comboF = cpool2.tile(
                        [128, 256], fp16, name="comboF", tag="comboF"
                    )
                    nc.scalar.activation(
                        comboF[0:64, :],
                        csa2[gsel][:, bass.ds(st0 + s, 256)],
                        Act.Copy,
                    )
                    nc.scalar.copy(comboF[64:128, :], dpb[64:128, :])
                else:
                    comboF = comboZ[si % 2]
                    nc.scalar.activation(
                        comboF[0:64, :],
                        csa2[gsel][:, bass.ds(st0 + s, 256)],
                        Act.Copy,
                    )
                nc.tensor.matmul(
                    dpa[0:64, :],
                    ssA,
                    comboF[:, :],
                    start=False,
                    stop=True,
                    tile_position=(0, 0),
                )
                e = e_bufs[si % 3]
                nc.scalar.activation(
                    e[0:64, :],
                    dpa[0:64, :],
                    Act.Exp,
                    bias=bias2[:, s : s + 1],
                    scale=-2.0,
                )
                nc.tensor.matmul(
                    ps_self,
                    ssA,
                    e[:, :],
                    start=(si == 0),
                    stop=(si == NSLOT - 1),
                )
                nc.tensor.matmul(
                    ps_pair[:, bass.ds(vp0 + s, 256)],
                    ssA,
                    e[:, :],
                    start=False,
                    stop=(si == NSLOT - 1),
                    skip_group_check=True,
                )

            for si, s in enumerate(order):
                recs.append(make_rec(si, s))

            # ---- Phase 1 ----
            # wraps: half 0 on DVE (psum src), half 1 on GPS (SBUF src).
            # mtb2 builds: GPS takes kh in [0,5) and the abs-path chunks
            # [16,19) of each half; ACT takes kh in [5,16).
            early_h0 = {9: 0, 12: 1, 15: 2}  # kop -> rec idx for h0 TTs
            for kop in range(NCH // 2):
                ko0 = 2 * kop
                if kop == 0:
                    tsb = tsb0
                else:
                    tsb = tpool.tile([128, 1024], fp16, tag="tsb")
                    nc.sync.dma_start(
                        tsb[:, :], tt_d[:, ko0 * 512 : (ko0 + 2) * 512]
                    )
                mp = mpsum.tile([128, 512], fp32, tag="mp")
                for k2 in range(2):
                    for cc in range(4):
                        nc.tensor.matmul(
                            mp[:, k2 * 256 : (k2 + 1) * 256],
                            tsb[:, (k2 * 4 + cc) * 128 : (k2 * 4 + cc + 1) * 128],
                            xt[:, cc * 256 : (cc + 1) * 256],
                            start=(cc == 0),
                            stop=(cc == 3),
                        )
                mp3 = mp[:, :].rearrange("p (k w) -> p k w", k=2)
                ko_pair = (ko0, ko0 + 1)
                if ko0 // NH == (ko0 + 1) // NH:
                    nc.scalar.copy(
                        mta3[:, ko0 : ko0 + 2, 0:256], mp3[:, :, :]
                    )
                else:
                    nc.scalar.copy(mta3[:, ko0, 0:256], mp3[:, 0, :])
                    nc.scalar.copy(mta3[:, ko0 + 1, 0:256], mp3[:, 1, :])
                for k2, ko in enumerate(ko_pair):
                    h, kh = divmod(ko, NH)
                    if h == 0:
                        nc.vector.tensor_copy(
                            mta3[:, ko, 256:385], mp3[:, k2, 0:129]
                        )
                    else:
                        nc.gpsimd.tensor_copy(
                            mta3[:, ko, 256:385], mta3[:, ko, 0:129]
                        )
                    if kh < 5 or kh >= ND2:
                        nc.gpsimd.tensor_copy(
                            mtb3[:, ko, :], mta3[:, ko, bass.ds(gt0, CWB)]
                        )
                    else:
                        nc.scalar.copy(
                            mtb3[:, ko, :], mta3[:, ko, bass.ds(st0, CWB)]
                        )
                if kop in early_h0:
                    emit_dve(recs[early_h0[kop]], 0)

            # ---- Phase 2 ----
            for k in range(NSLOT):
                emit_gps(recs[k], 0)
                emit_dve(recs[k], 1)
                emit_gps(recs[k], 1)
                emit_abs(recs[k], 0)
                emit_abs(recs[k], 1)
                if k + 3 < NSLOT:
                    emit_dve(recs[k + 3], 0)
                emit_pe(recs[k])

            pairsb = cpool.tile([64, 512], fp32)
            nc.scalar.copy(pairsb[:, :], ps_pair[:, :])
            outsb = cpool.tile([64, 256], fp32)
            nc.vector.tensor_tensor(
                outsb[:, :], pairsb[:, 0:256], pairsb[:, 256:512], Alu.add
            )
            nc.vector.tensor_tensor(
                outsb[:, :], outsb[:, :], ps_self, Alu.add
            )
            nc.sync.dma_start(out_d[:, :], outsb[:, :])

    nc.compile()
    return nc


def get_nc():
    if "nc" not in _NC_CACHE:
        _NC_CACHE["nc"] = _build_nc()
    return _NC_CACHE["nc"]


def _k_sets():
    """k indices covered by the DVE (max-identity) chunks vs abs chunks."""
    dve_chunks = list(range(0, NDVE)) + list(range(NH, NH + NDVE))
    ks = []
    for c in dve_chunks:
        ks += [2 * c, 2 * c + 1]
    return sorted(k for k in ks if k < K)


def host_inputs(x, T):
    """Host-side shard prep: returns the 8 per-core input maps."""
    x = np.asarray(x, dtype=np.float32)
    T = np.asarray(T, dtype=np.float32)
    T2p = np.zeros((F, KOP), np.float32)
    T2p[:, :KO] = T.reshape(F, KO)
    # tt[p, ko*512 + cc*128 + j] = T2p[cc*128+p, ko*128+j]
    tt = (
        np.ascontiguousarray(
            T2p.reshape(4, 128, NCH, 128).transpose(1, 2, 0, 3)
        )
        .reshape(128, NCH * 512)
        .astype(np.float16)
    )
    # xt[p, cc*256 + i] = x[i, cc*128+p]
    xt = (
        np.ascontiguousarray(x.T.reshape(4, 128, B).transpose(1, 0, 2))
        .reshape(128, 1024)
        .astype(np.float16)
    )
    # tts2: partial (DVE k-set) then full presummed T
    T3 = T.reshape(F, K, O)
    kset = _k_sets()
    TS_part = T3[:, kset, :].sum(axis=1)
    TS_full = T3.sum(axis=1)

    def pack_ts(TS):
        return (
            np.ascontiguousarray(TS.reshape(4, 128, O).transpose(1, 0, 2))
            .reshape(128, 256)
            .astype(np.float16)
        )

    tts2 = np.concatenate([pack_ts(TS_part), pack_ts(TS_full)], axis=1)
    ssA = (np.arange(128)[:, None] % 64 == np.arange(64)[None, :]).astype(
        np.float16
    )
    ss2 = np.concatenate([ssA, 0.5 * ssA], axis=1).astype(np.float16)
    in_maps = []
    for c in range(8):
        offs = np.array([[16 * c + 1, 0]], np.int32)
        biases = np.zeros((64, NSLOT), np.float32)
        if c == 7:
            biases[:, 15] = -LN2  # t = 128: every pair covered twice
        in_maps.append(
            {
                "xt": xt,
                "tt": tt,
                "tts2": tts2,
                "ssel": ss2,
                "offs": offs,
                "bias2": biases,
            }
        )
    return in_maps


def combine(results):
    """Sum per-core partial outputs [64,256] -> full [256,64] fp32.

    The reference computes sum_j exp(-d) (including the j=i term, = 1.0) in
    fp32 and then subtracts 1.0. Replicate those fp32 semantics exactly: the
    off-diagonal terms here are ~1e-25 and are fully absorbed by the +1.
    """
    acc = np.zeros((64, 256), np.float64)
    for r in results:
        acc += r["out"].astype(np.float64)
    full = np.ascontiguousarray(acc.T).astype(np.float32)
    return (np.float32(1.0) + full) - np.float32(1.0)


def run_on_hw(x, T, trace=False):
    from concourse.bass_utils import run_bass_kernel_spmd

    nc = get_nc()
    in_maps = host_inputs(x, T)
    res = run_bass_kernel_spmd(
        nc, in_maps, core_ids=list(range(8)), trace=trace
    )
    return combine(res.results), res


def kernel(x, T):
    out, _ = run_on_hw(x, T, trace=False)
    return out


# revision 12
# speedup vs baseline: 1.0629x; 1.0629x over previous
"""MinibatchDiscrimination Bass kernel for 8 TRN2 NeuronCores.

out[i,o] = sum_{j!=i} exp(-sum_k |M[i,k,o]-M[j,k,o]|),  M = x @ T.

Cyclic-offset pairing: shift t pairs row i with row (i+t) mod B; core c
computes t in [16c+1, 16c+16] (t=128 halved via an exp bias of -ln2),
covering every unordered pair exactly once. The L1 distance uses
    sum_k |a-b| = 2*sum_k max(a,b) - sum_k a - sum_k b.

Engine plan (v2.2). The DVE is the only max-capable engine (Pool has
no max ALU here, and any GPS SBUF traffic throttles the DVE's 2-port
fp16 2x stream), so phase 2 is DVE-paced: one [128,19,256] max per
half per slot. Everything else hides under it:
  - PE reduces the 38 chunks plus two "combo" chunks (row-sum
    corrections via csa = -0.5*sa riding in rows 0-63 / 64-127 of a
    [128,256] rhs) with one selector weight load for all of phase 2;
    e rows 64-127 stay zero so the e-accumulation matmuls use the
    same selector. Row-sums come from 4 matmuls against the
    host-presummed T. A PE warm-up burst during the DMA lead-in keeps
    HAM at 8/8 through phase 1.
  - ACT copies psum->mta and builds mtb2 = M^T shifted by t0 (272
    wide, dynamic-offset reads) so odd-t DVE reads are 4B-aligned
    static; even-t reads use a dynamic aligned offset of mta.
  - DVE also copies the cyclic-wrap region (psum-sourced) and runs
    the first four slots' half-0 maxes inside phase 1.
"""

import numpy as np

B = 256
F = 512
K = 75
O = 64
KO = K * O          # 4800
KOP = 4864          # padded to 38*128
NCH = KOP // 128    # 38 ko-chunks
NH = NCH // 2       # 19 chunks per half
CWA = 388           # mta chunk width (256 + 129 wrap + pad)
CWB = 272           # mtb2 chunk width: M^T[.., t0 + j], j in [0,272)
NSLOT = 16
LN2 = float(np.log(2.0))

_NC_CACHE = {}


def _build_nc():
    import concourse.bacc as bacc
    import concourse.bass as bass
    import concourse.mybir as mybir
    from concourse import tile

    fp16 = mybir.dt.float16
    fp32 = mybir.dt.float32
    bf16 = mybir.dt.bfloat16
    i32 = mybir.dt.int32
    Alu = mybir.AluOpType
    Act = mybir.ActivationFunctionType

    nc = bacc.Bacc(
        "TRN2", target_bir_lowering=False, debug=False, num_devices=8
    )

    with tile.TileContext(nc) as tc:
        xt_d = nc.dram_tensor("xt", [128, 1024], fp16, kind="ExternalInput")
        tt_d = nc.dram_tensor("tt", [128, NCH * 512], fp16, kind="ExternalInput")
        ts_d = nc.dram_tensor("tts2", [128, 512], fp16, kind="ExternalInput")
        ss_d = nc.dram_tensor("ssel", [128, 128], fp16, kind="ExternalInput")
        of_d = nc.dram_tensor("offs", [1, 2], i32, kind="ExternalInput")
        bi_d = nc.dram_tensor("bias2", [64, NSLOT], fp32, kind="ExternalInput")
        out_d = nc.dram_tensor("out", [64, 256], fp32, kind="ExternalOutput")

        with (
            tc.tile_pool(name="const", bufs=1) as cpool,
            tc.tile_pool(name="tload", bufs=3) as tpool,
            tc.tile_pool(name="mx", bufs=4) as mxpool,
            tc.tile_pool(name="cmb", bufs=2) as cpool2,
            tc.tile_pool(name="mpsum", bufs=2, space="PSUM") as mpsum,
            tc.tile_pool(name="dpa", bufs=3, space="PSUM") as dpapool,
            tc.tile_pool(name="apsum", bufs=1, space="PSUM") as apsum,
        ):
            # small inputs first so the PE warm-up can start early
            ss2 = cpool.tile([128, 128], fp16)
            nc.sync.dma_start(ss2[:, :], ss_d[:, :])
            offs = cpool.tile([1, 2], i32)
            nc.sync.dma_start(offs[:, :], of_d[:, :])
            bias2 = cpool.tile([64, NSLOT], fp32)
            nc.sync.dma_start(bias2[:, :], bi_d[:, :])
            tts2 = cpool.tile([128, 512], fp16)
            nc.sync.dma_start(tts2[:, :], ts_d[:, :])
            xt = cpool.tile([128, 1024], fp16)
            for cc in range(4):
                nc.sync.dma_start(
                    xt[:, cc * 256 : (cc + 1) * 256],
                    xt_d[:, cc * 256 : (cc + 1) * 256],
                )
            tsb0 = tpool.tile([128, 1024], fp16, tag="tsb")
            nc.sync.dma_start(tsb0[:, :], tt_d[:, 0:1024])

            ssA = ss2[:, 0:64]  # 0/1 selector

            # single M^T tile (subtile deps track chunk completion)
            mta = cpool.tile([128, NCH * CWA], fp16, name="mta", tag="mta")
            mtb = cpool.tile([128, NCH * CWB], fp16, name="mtb", tag="mtb")
            mta3 = mta[:, :].rearrange("p (c w) -> p c w", w=CWA)
            mtb3 = mtb[:, :].rearrange("p (c w) -> p c w", w=CWB)

            # acc bank: sa (early) + ps_self (late) share one psum bank
            acc = apsum.tile([64, 512], fp32, tag="acc")
            sa_ps = acc[:, 0:256]
            ps_self = acc[:, 256:512]
            ps_pair = apsum.tile([64, 512], fp32, tag="ppair")
            nc.vector.memset(ps_pair[:, :], 0.0)

            # ---- PE warm-up during the DMA lead-in (HAM -> 8/8) ----
            warm = mpsum.tile([128, 512], fp32, tag="mp")
            for w in range(64):
                nc.tensor.matmul(
                    warm[0:64, 0:64],
                    ssA,
                    ssA,
                    start=(w == 0),
                    stop=(w == 63),
                )

            # row-sums from host-presummed T: sa[o,i] = sum_f TS[f,o]x[i,f]
            tts3 = tts2[:, :].rearrange("p (g c w) -> p g c w", g=2, w=64)
            for cc in range(4):
                nc.tensor.matmul(
                    sa_ps,
                    tts3[:, 1, cc, :],
                    xt[:, cc * 256 : (cc + 1) * 256],
                    start=(cc == 0),
                    stop=(cc == 3),
                )
            csa2 = cpool.tile([64, 512], fp16)
            nc.scalar.activation(csa2[:, 0:256], sa_ps, Act.Copy, scale=-0.5)
            nc.scalar.activation(csa2[:, 256:512], sa_ps, Act.Copy, scale=-0.5)

            # static combo chunk (rows 0-63 = csa_i, rows 64-127 = 0)
            comboS = cpool.tile([128, 256], fp16)
            nc.vector.memset(comboS[64:128, :], 0.0)
            nc.scalar.copy(comboS[0:64, :], csa2[:, 0:256])
            # comboZ: per-slot dynamic corr_j (rows 64-127 stay 0)
            comboZ = []
            for z in range(2):
                cb = cpool.tile([128, 256], fp16, name=f"comboZ_{z}")
                nc.vector.memset(cb[64:128, :], 0.0)
                comboZ.append(cb)

            # e tiles: rows 64-127 stay zero forever (selector e-acc)
            e_bufs = [cpool.tile([128, 256], bf16, name=f"e{i}") for i in range(3)]
            for eb in e_bufs:
                nc.vector.memset(eb[64:128, :], 0.0)

            # t0 registers per engine
            rtv = nc.vector.alloc_register("t0v")
            nc.vector.reg_load(rtv, offs[0:1, 0:1])
            vt0 = nc.vector.snap(rtv, donate=True, min_val=1, max_val=113)
            rts = nc.scalar.alloc_register("t0s")
            nc.scalar.reg_load(rts, offs[0:1, 0:1])
            st0 = nc.scalar.snap(rts, donate=True, min_val=1, max_val=113)
            rtp = nc.tensor.alloc_register("t0p")
            nc.tensor.reg_load(rtp, offs[0:1, 0:1])
            vp0 = nc.tensor.snap(rtp, donate=True, min_val=1, max_val=113)

            order = [s for s in range(NSLOT) if s % 2 == 1] + [
                s for s in range(NSLOT) if s % 2 == 0
            ]
            recs = []

            def make_rec(si, s):
                mx = mxpool.tile([128, NCH * 256], fp16, name="mx", tag="mx")
                m3 = mx[:, :].rearrange("p (c w) -> p c w", w=256)
                return (si, s, m3)

            def emit_dve(rec, h):
                si, s, m3 = rec
                c0 = h * NH
                if s % 2 == 0:  # odd t: static aligned read of mtb2
                    src = mtb3[:, c0 : c0 + NH, s : s + 256]
                else:  # even t: dynamic aligned read of mta
                    src = mta3[:, c0 : c0 + NH, bass.ds(vt0 + s, 256)]
                nc.vector.tensor_tensor(
                    m3[:, c0 : c0 + NH, :],
                    mta3[:, c0 : c0 + NH, 0:256],
                    src,
                    Alu.max,
                )

            def emit_pe(rec):
                si, s, m3 = rec
                dpa_t = dpapool.tile([128, 512], fp32, name="dpa_t", tag="dpa")
                dpa = dpa_t[:, 0:256]
                for c in range(NCH):
                    nc.tensor.matmul(
                        dpa[0:64, :],
                        ssA,
                        m3[:, c, :],
                        start=(c == 0),
                        stop=False,
                    )
                nc.tensor.matmul(
                    dpa[0:64, :],
                    ssA,
                    comboS[:, :],
                    start=False,
                    stop=False,
                )
                comboF = comboZ[si % 2]
                nc.scalar.activation(
                    comboF[0:64, :],
                    csa2[:, bass.ds(st0 + s, 256)],
                    Act.Copy,
                )
                nc.tensor.matmul(
                    dpa[0:64, :],
                    ssA,
                    comboF[:, :],
                    start=False,
                    stop=True,
                )
                e = e_bufs[si % 3]
                nc.scalar.activation(
                    e[0:64, :],
                    dpa[0:64, :],
                    Act.Exp,
                    bias=bias2[:, s : s + 1],
                    scale=-2.0,
                )
                nc.tensor.matmul(
                    ps_self,
                    ssA,
                    e[:, :],
                    start=(si == 0),
                    stop=(si == NSLOT - 1),
                )
                nc.tensor.matmul(
                    ps_pair[:, bass.ds(vp0 + s, 256)],
                    ssA,
                    e[:, :],
                    start=False,
                    stop=(si == NSLOT - 1),
                    skip_group_check=True,
                )

            for si, s in enumerate(order):
                recs.append(make_rec(si, s))

            # ---- Phase 1 ----
            early_h0 = {9: 0, 11: 1, 13: 2, 15: 3}  # kop -> rec for h0 TT
            for kop in range(NCH // 2):
                ko0 = 2 * kop
                if kop == 0:
                    tsb = tsb0
                else:
                    tsb = tpool.tile([128, 1024], fp16, tag="tsb")
                    nc.sync.dma_start(
                        tsb[:, :], tt_d[:, ko0 * 512 : (ko0 + 2) * 512]
                    )
                mp = mpsum.tile([128, 512], fp32, tag="mp")
                for k2 in range(2):
                    for cc in range(4):
                        nc.tensor.matmul(
                            mp[:, k2 * 256 : (k2 + 1) * 256],
                            tsb[:, (k2 * 4 + cc) * 128 : (k2 * 4 + cc + 1) * 128],
                            xt[:, cc * 256 : (cc + 1) * 256],
                            start=(cc == 0),
                            stop=(cc == 3),
                        )
                mp3 = mp[:, :].rearrange("p (k w) -> p k w", k=2)
                if ko0 // NH == (ko0 + 1) // NH:
                    nc.scalar.copy(mta3[:, ko0 : ko0 + 2, 0:256], mp3[:, :, :])
                else:
                    nc.scalar.copy(mta3[:, ko0, 0:256], mp3[:, 0, :])
                    nc.scalar.copy(mta3[:, ko0 + 1, 0:256], mp3[:, 1, :])
                for k2, ko in enumerate((ko0, ko0 + 1)):
                    # cyclic-wrap copy on DVE (psum-sourced: no port clash)
                    nc.vector.tensor_copy(
                        mta3[:, ko, 256:385], mp3[:, k2, 0:129]
                    )
                    # narrow t0-shifted copy on ACT
                    nc.scalar.copy(
                        mtb3[:, ko, :], mta3[:, ko, bass.ds(st0, CWB)]
                    )
                if kop in early_h0:
                    emit_dve(recs[early_h0[kop]], 0)

            # ---- Phase 2 ----
            for k in range(NSLOT):
                emit_dve(recs[k], 1)
                if k + 4 < NSLOT:
                    emit_dve(recs[k + 4], 0)
                emit_pe(recs[k])

            pairsb = cpool.tile([64, 512], fp32)
            nc.scalar.copy(pairsb[:, :], ps_pair[:, :])
            outsb = cpool.tile([64, 256], fp32)
            nc.vector.tensor_tensor(
                outsb[:, :], pairsb[:, 0:256], pairsb[:, 256:512], Alu.add
            )
            nc.vector.tensor_tensor(
                outsb[:, :], outsb[:, :], ps_self, Alu.add
            )
            nc.sync.dma_start(out_d[:, :], outsb[:, :])

    nc.compile()
    return nc


def get_nc():
    if "nc" not in _NC_CACHE:
        _NC_CACHE["nc"] = _build_nc()
    return _NC_CACHE["nc"]


def host_inputs(x, T):
    """Host-side shard prep: returns the 8 per-core input maps."""
    x = np.asarray(x, dtype=np.float32)
    T = np.asarray(T, dtype=np.float32)
    T2p = np.zeros((F, KOP), np.float32)
    T2p[:, :KO] = T.reshape(F, KO)
    # tt[p, ko*512 + cc*128 + j] = T2p[cc*128+p, ko*128+j]
    tt = (
        np.ascontiguousarray(
            T2p.reshape(4, 128, NCH, 128).transpose(1, 2, 0, 3)
        )
        .reshape(128, NCH * 512)
        .astype(np.float16)
    )
    # xt[p, cc*256 + i] = x[i, cc*128+p]
    xt = (
        np.ascontiguousarray(x.T.reshape(4, 128, B).transpose(1, 0, 2))
        .reshape(128, 1024)
        .astype(np.float16)
    )
    # tts2: [partial (unused in v2.2), full] presummed T
    T3 = T.reshape(F, K, O)
    TS_full = T3.sum(axis=1)

    def pack_ts(TS):
        return (
            np.ascontiguousarray(TS.reshape(4, 128, O).transpose(1, 0, 2))
            .reshape(128, 256)
            .astype(np.float16)
        )

    p = pack_ts(TS_full)
    tts2 = np.concatenate([p, p], axis=1)
    ssA = (np.arange(128)[:, None] % 64 == np.arange(64)[None, :]).astype(
        np.float16
    )
    ss2 = np.concatenate([ssA, 0.5 * ssA], axis=1).astype(np.float16)
    in_maps = []
    for c in range(8):
        offs = np.array([[16 * c + 1, 0]], np.int32)
        biases = np.zeros((64, NSLOT), np.float32)
        if c == 7:
            biases[:, 15] = -LN2  # t = 128: every pair covered twice
        in_maps.append(
            {
                "xt": xt,
                "tt": tt,
                "tts2": tts2,
                "ssel": ss2,
                "offs": offs,
                "bias2": biases,
            }
        )
    return in_maps


def combine(results):
    """Sum per-core partial outputs [64,256] -> full [256,64] fp32.

    The reference computes sum_j exp(-d) (including the j=i term, = 1.0) in
    fp32 and then subtracts 1.0. Replicate those fp32 semantics exactly: the
    off-diagonal terms here are ~1e-25 and are fully absorbed by the +1.
    """
    acc = np.zeros((64, 256), np.float64)
    for r in results:
        acc += r["out"].astype(np.float64)
    full = np.ascontiguousarray(acc.T).astype(np.float32)
    return (np.float32(1.0) + full) - np.float32(1.0)


def run_on_hw(x, T, trace=False):
    from concourse.bass_utils import run_bass_kernel_spmd

    nc = get_nc()
    in_maps = host_inputs(x, T)
    res = run_bass_kernel_spmd(
        nc, in_maps, core_ids=list(range(8)), trace=trace
    )
    return combine(res.results), res


def kernel(x, T):
    out, _ = run_on_hw(x, T, trace=False)
    return out


# revision 15
# speedup vs baseline: 1.2169x; 1.1448x over previous
"""MinibatchDiscrimination Bass kernel for 8 TRN2 NeuronCores.

out[i,o] = sum_{j!=i} exp(-sum_k |M[i,k,o]-M[j,k,o]|),  M = x @ T.

Strategy: the BxB distance matrix is symmetric. Cyclic-offset pairing:
shift t pairs row i with row (i+t) mod B; t=1..B/2 covers every unordered
pair exactly once (t=B/2 covers each twice -> halved via an exp bias of
-ln2). Core c computes shifts t in [16c+1, 16c+16]; each pair contributes
to both of its rows. Host sums the 8 partial outputs and applies the
reference's fp32 `(1 + s) - 1` absorption.

Key identity (avoids any elementwise |.| pass):
    sum_k |a_k - b_k| = 2*sum_k max(a_k, b_k) - sum_k a_k - sum_k b_k
so per shift only ONE DVE max pass feeds the PE; the row-sum corrections
are folded into the same PSUM accumulation as two fp32 matmuls vs -0.5*I.

M^T tiles are split into chunk-halves so the first shift slots start as
soon as half 1 exists, overlapping the rest of the M matmul. t's parity
equals the slot parity on every core (t = 16c + s + 1), so even/odd-shift
layout choices are compile-time; all per-core values derive from one
register load of t0 = 16c + 1.
"""

import numpy as np

B = 256
F = 512
K = 75
O = 64
KO = K * O          # 4800
KOP = 4864          # padded to 38*128
NCH = KOP // 128    # 38 ko-chunks
NH = NCH // 2       # 19 chunks per half
CWA = 384           # chunk width: M^T[.., i] doubled to i in [0,384)
NSLOT = 16
LN2 = float(np.log(2.0))

_NC_CACHE = {}


def _build_nc():
    import concourse.bacc as bacc
    import concourse.bass as bass
    import concourse.mybir as mybir
    from concourse import tile

    fp16 = mybir.dt.float16
    fp32 = mybir.dt.float32
    bf16 = mybir.dt.bfloat16
    i32 = mybir.dt.int32
    Alu = mybir.AluOpType
    Act = mybir.ActivationFunctionType

    nc = bacc.Bacc(
        "TRN2", target_bir_lowering=False, debug=False, num_devices=8
    )

    with tile.TileContext(nc) as tc:
        xt_d = nc.dram_tensor("xt", [128, 1024], fp16, kind="ExternalInput")
        tt_d = nc.dram_tensor("tt", [128, NCH * 512], fp16, kind="ExternalInput")
        ss_d = nc.dram_tensor("ssel", [128, 64], fp16, kind="ExternalInput")
        nh_d = nc.dram_tensor("nhalf", [64, 64], fp16, kind="ExternalInput")
        id_d = nc.dram_tensor("ident", [64, 64], bf16, kind="ExternalInput")
        of_d = nc.dram_tensor("offs", [1, 2], i32, kind="ExternalInput")
        bi_d = nc.dram_tensor("bias", [64, NSLOT], fp32, kind="ExternalInput")
        out_d = nc.dram_tensor("out", [64, 256], fp32, kind="ExternalOutput")

        with (
            tc.tile_pool(name="const", bufs=1) as cpool,
            tc.tile_pool(name="tload", bufs=3) as tpool,
            tc.tile_pool(name="mxp", bufs=4) as dpool,
            tc.tile_pool(name="esb", bufs=10) as epool,
            tc.tile_pool(name="mpsum", bufs=2, space="PSUM") as mpsum,
            tc.tile_pool(name="dpsum", bufs=2, space="PSUM") as dpsum,
            tc.tile_pool(name="apsum", bufs=1, space="PSUM") as apsum,
        ):
            # small selector first: it feeds the PE warm-up below
            ss = cpool.tile([128, 64], fp16)
            nc.sync.dma_start(ss[:, :], ss_d[:, :])
            # prefetch the first T chunk-pair before anything else so the
            # first matmul can start as early as possible
            tsb0 = tpool.tile([128, 1024], fp16, tag="tsb")
            nc.sync.dma_start(tsb0[:, :], tt_d[:, 0:1024])
            xt = cpool.tile([128, 1024], fp16)
            for cc in range(4):
                nc.sync.dma_start(
                    xt[:, cc * 256 : (cc + 1) * 256],
                    xt_d[:, cc * 256 : (cc + 1) * 256],
                )
            nh = cpool.tile([64, 64], fp16)
            nc.sync.dma_start(nh[:, :], nh_d[:, :])
            ident = cpool.tile([64, 64], bf16)
            nc.sync.dma_start(ident[:, :], id_d[:, :])
            offs = cpool.tile([1, 2], i32)
            nc.sync.dma_start(offs[:, :], of_d[:, :])
            bias = cpool.tile([64, NSLOT], fp32)
            nc.sync.dma_start(bias[:, :], bi_d[:, :])

            # chunk-half tiles: [0]=chunks 0..18, [1]=chunks 19..37
            mta = [
                cpool.tile([128, NH * CWA], fp16, name=f"mta{h}", tag=f"mta{h}")
                for h in (0, 1)
            ]
            mtb = [
                cpool.tile([128, NH * CWA], fp16, name=f"mtb{h}", tag=f"mtb{h}")
                for h in (0, 1)
            ]
            # e accumulators live in PSUM, fed by fp32 identity matmuls
            ps_self = apsum.tile([64, 256], fp32, tag="pself")
            ps_pair = apsum.tile([64, 512], fp32, tag="ppair")
            nc.vector.memset(ps_pair[:, :], 0.0)

            mta3 = [t[:, :].rearrange("p (c w) -> p c w", w=CWA) for t in mta]
            mtb3 = [t[:, :].rearrange("p (c w) -> p c w", w=CWA) for t in mtb]
            sa_ps = apsum.tile([64, 256], fp32, tag="sa")

            # PE warm-up burst during the DMA lead-in: ~64 tiny matmuls
            # of sustained activity flip HAM to K=8/8 before the real
            # phase-1 matmuls arrive (they otherwise run 2x slow-clocked
            # for the first ~23us).
            warm = mpsum.tile([128, 512], fp32, tag="mp")
            for w in range(64):
                nc.tensor.matmul(
                    warm[0:64, 0:64],
                    ss[:, 0:64],
                    ss[:, 0:64],
                    start=(w == 0),
                    stop=(w == 63),
                )

            # Phase 1: MTa = M^T in (ko-chunk, i) layout, i doubled to 384;
            # MTb = same shifted by one i (for odd shifts' 4B alignment).
            # Two ko-chunks per psum tile; Sa row-sum matmuls interleaved so
            # sa2 is ready the moment the last chunk lands.
            for kop in range(NCH // 2):
                ko0 = 2 * kop
                if kop == 0:
                    tsb = tsb0
                else:
                    tsb = tpool.tile([128, 1024], fp16, tag="tsb")
                    nc.sync.dma_start(
                        tsb[:, :], tt_d[:, ko0 * 512 : (ko0 + 2) * 512]
                    )
                mp = mpsum.tile([128, 512], fp32)
                for k2 in range(2):
                    for cc in range(4):
                        nc.tensor.matmul(
                            mp[:, k2 * 256 : (k2 + 1) * 256],
                            tsb[:, (k2 * 4 + cc) * 128 : (k2 * 4 + cc + 1) * 128],
                            xt[:, cc * 256 : (cc + 1) * 256],
                            start=(cc == 0),
                            stop=(cc == 3),
                        )
                mp3 = mp[:, :].rearrange("p (k w) -> p k w", k=2)
                for k2 in range(2):
                    ko = ko0 + k2
                    h, kh = divmod(ko, NH)
                    ba = kh * CWA
                    nc.scalar.copy(mta[h][:, ba : ba + 256], mp3[:, k2, :])
                    nc.scalar.copy(
                        mta[h][:, ba + 256 : ba + 384], mp3[:, k2, 0:128]
                    )
                    # Sa accumulation for this chunk
                    nc.tensor.matmul(
                        sa_ps[:, :],
                        ss[:, 0:64],
                        mta3[h][:, kh, 0:256],
                        start=(ko == 0),
                        stop=(ko == NCH - 1),
                    )
                    # MTb[ko, 0:382] = MTa[ko, 1:383]. Half 0 on GPSIMD
                    # (free all of phase 1); half 1 on ACT (near-idle during
                    # phase 2). Both only gate the odd-shift slots, which
                    # run last.
                    if h == 0:
                        nc.gpsimd.tensor_copy(
                            mtb[h][:, ba : ba + 382], mta[h][:, ba + 1 : ba + 383]
                        )

            sa2 = cpool.tile([64, 512], fp16)
            nc.scalar.copy(sa2[:, 0:256], sa_ps[:, :])
            nc.scalar.copy(sa2[:, 256:512], sa_ps[:, :])

            # MTb half 1 on ACT, overlapping the even-shift slots
            for kh in range(NH):
                ba = kh * CWA
                nc.scalar.copy(
                    mtb[1][:, ba : ba + 382], mta[1][:, ba + 1 : ba + 383]
                )

            # one register load of t0 = 16*core + 1 per engine; everything
            # else is t0 + compile-time constants.
            rtv = nc.vector.alloc_register("t0v")
            nc.vector.reg_load(rtv, offs[0:1, 0:1])
            vt0 = nc.vector.snap(rtv, donate=True, min_val=1, max_val=113)
            rtp = nc.tensor.alloc_register("t0p")
            nc.tensor.reg_load(rtp, offs[0:1, 0:1])
            vp0 = nc.tensor.snap(rtp, donate=True, min_val=1, max_val=113)

            # Phase 2, per shift slot s (t = t0 + s):
            #   DVE max (fp16 2x, 4 instrs: 2 chunk-halves x 2 i-blocks)
            #   -> PE: 38 chunk matmuls + 2 fp32 corrections into one
            #   [64,256] psum = d/2 -> ACT exp(scale=-2, bias) ->
            #   acc adds (self on GPSIMD, pair on DVE).
            # Even-t slots (odd s) run first: they only need MTa.
            order = [s for s in range(NSLOT) if s % 2 == 1] + [
                s for s in range(NSLOT) if s % 2 == 0
            ]
            for si, s in enumerate(order):
                par = (s + 1) % 2  # t parity; even t -> MTa, odd t -> MTb
                src3 = mta3 if par == 0 else mtb3
                mx = dpool.tile([128, NCH * 256], fp16)
                m3 = mx[:, :].rearrange("p (c w) -> p c w", w=256)
                for h in (0, 1):
                    for blk in (0, 1):
                        off = vt0 + (s + blk * 128 - par)
                        nc.vector.tensor_tensor(
                            m3[:, h * NH : (h + 1) * NH, blk * 128 : (blk + 1) * 128],
                            mta3[h][:, :, blk * 128 : (blk + 1) * 128],
                            src3[h][:, :, bass.ds(off, 128)],
                            Alu.max,
                        )
                dp = dpsum.tile([64, 256], fp32, tag="dp")
                for c in range(NCH):
                    nc.tensor.matmul(
                        dp[:, :],
                        ss[:, 0:64],
                        m3[:, c, :],
                        start=(c == 0),
                        stop=False,
                    )
                nc.tensor.matmul(
                    dp[:, :], nh[:, :], sa2[:, 0:256], start=False, stop=False
                )
                nc.tensor.matmul(
                    dp[:, :],
                    nh[:, :],
                    sa2[:, bass.ds(vp0 + s, 256)],
                    start=False,
                    stop=True,
                )
                e = epool.tile([64, 256], bf16, tag="e")
                nc.scalar.activation(
                    e[:, :], dp[:, :], Act.Exp, bias=bias[:, s : s + 1], scale=-2.0
                )
                # accumulate e on the PE: self into a fixed [64,256] window,
                # pair into a dynamic window of the pre-zeroed [64,512] bank
                nc.tensor.matmul(
                    ps_self[:, :],
                    ident[:, :],
                    e[:, :],
                    start=(si == 0),
                    stop=(si == NSLOT - 1),
                )
                nc.tensor.matmul(
                    ps_pair[:, bass.ds(vp0 + s, 256)],
                    ident[:, :],
                    e[:, :],
                    start=False,
                    stop=(si == NSLOT - 1),
                    skip_group_check=True,
                )

            pairsb = cpool.tile([64, 512], fp32)
            nc.scalar.copy(pairsb[:, :], ps_pair[:, :])
            outsb = cpool.tile([64, 256], fp32)
            nc.vector.tensor_tensor(
                outsb[:, :], pairsb[:, 0:256], pairsb[:, 256:512], Alu.add
            )
            nc.vector.tensor_tensor(
                outsb[:, :], outsb[:, :], ps_self[:, :], Alu.add
            )
            nc.sync.dma_start(out_d[:, :], outsb[:, :])

    nc.compile()
    return nc


def get_nc():
    if "nc" not in _NC_CACHE:
        _NC_CACHE["nc"] = _build_nc()
    return _NC_CACHE["nc"]


def host_inputs(x, T):
    """Host-side shard prep: returns the 8 per-core input maps."""
    x = np.asarray(x, dtype=np.float32)
    T = np.asarray(T, dtype=np.float32)
    T2p = np.zeros((F, KOP), np.float32)
    T2p[:, :KO] = T.reshape(F, KO)
    # tt[p, ko*512 + cc*128 + j] = T2p[cc*128+p, ko*128+j]
    tt = (
        np.ascontiguousarray(
            T2p.reshape(4, 128, NCH, 128).transpose(1, 2, 0, 3)
        )
        .reshape(128, NCH * 512)
        .astype(np.float16)
    )
    # xt[p, cc*256 + i] = x[i, cc*128+p]
    xt = (
        np.ascontiguousarray(x.T.reshape(4, 128, B).transpose(1, 0, 2))
        .reshape(128, 1024)
        .astype(np.float16)
    )
    ss = (np.arange(128)[:, None] % 64 == np.arange(64)[None, :]).astype(
        np.float16
    )
    import ml_dtypes
    nh = (-0.5 * np.eye(64)).astype(np.float16)
    ident = np.eye(64).astype(ml_dtypes.bfloat16)
    in_maps = []
    for c in range(8):
        offs = np.array([[16 * c + 1, 0]], np.int32)
        biases = np.zeros((64, NSLOT), np.float32)
        if c == 7:
            biases[:, 15] = -LN2  # t = 128: every pair covered twice
        in_maps.append(
            {
                "xt": xt,
                "tt": tt,
                "ssel": ss,
                "nhalf": nh,
                "ident": ident,
                "offs": offs,
                "bias": biases,
            }
        )
    return in_maps


def combine(results):
    """Sum per-core partial outputs [64,256] -> full [256,64] fp32.

    The reference computes sum_j exp(-d) (including the j=i term, = 1.0) in
    fp32 and then subtracts 1.0. Replicate those fp32 semantics exactly: the
    off-diagonal terms here are ~1e-25 and are fully absorbed by the +1.
    """
    acc = np.zeros((64, 256), np.float64)
    for r in results:
        acc += r["out"].astype(np.float64)
    full = np.ascontiguousarray(acc.T).astype(np.float32)
    return (np.float32(1.0) + full) - np.float32(1.0)


def run_on_hw(x, T, trace=False):
    from concourse.bass_utils import run_bass_kernel_spmd

    nc = get_nc()
    in_maps = host_inputs(x, T)
    res = run_bass_kernel_spmd(
        nc, in_maps, core_ids=list(range(8)), trace=trace
    )
    return combine(res.results), res


def kernel(x, T):
    out, _ = run_on_hw(x, T, trace=False)
    return out

